# revision 1
# baseline (speedup 1.0000x reference)
"""Distributed brute-force kNN retrieval (cosine similarity) on 8 Trainium2 cores.

Strategy (per spec sharding hint, adapted):
  - Shard the feature bank along N across 8 cores (62500 rows each).
  - Host pre-transposes each shard to [768, 62500] so the device can DMA
    contraction-major tiles directly (no on-device transpose needed).
  - Each core computes raw dot products q @ f_shard.T with float32r matmuls
    (PE-native, ~1e-4 relative error) and extracts the top-8 candidates per
    query per 8000-feature block with the DVE Max8/MaxIndex instructions.
  - Host gathers the 8*64 candidates per (query, core), rescores them exactly
    in fp32 (normalized cosine similarity, same math as the reference), does
    the final top-k reduction, and gathers the data segments.

The candidate margin (top-8 of every 8000-feature block when only the global
top-5 is needed) makes the device pass insensitive to the reduced-precision
matmul: a true top-5 entry would have to be pushed below rank 8 *within its
own block* by ~1e-2-magnitude noise on dot gaps that are ~1e0 apart.
"""

import numpy as np

import concourse.bacc as bacc
import concourse.mybir as mybir
from concourse.tile import TileContext
from concourse.bass_utils import run_bass_kernel_spmd

# Problem geometry (hardcoded per spec).
B = 64            # queries
D = 768           # feature dim
N = 500000        # feature rows
NCORES = 8
NSH = N // NCORES  # 62500 rows per core
KC = D // 128      # 6 contraction chunks of 128
CHUNK = 500        # matmul moving free dim (one PSUM bank, >=256 keeps fp32r fast)
NCHUNKS = NSH // CHUNK          # 125
GRP = 2                         # chunks per DMA group (3 MB per DMA)
BLOCK_CHUNKS = 16               # chunks per Max8 block (8000 features)
NBLOCKS = (NCHUNKS + BLOCK_CHUNKS - 1) // BLOCK_CHUNKS  # 8 (7 full + 13-chunk tail)
TOPB = 8                        # Max8 output width per block

_COMPILED = None
LAST_RESULTS = None  # test harness introspection


def _build():
    nc = bacc.Bacc("TRN2", target_bir_lowering=False, debug=False)
    qT = nc.declare_dram_parameter("qT", [D, B], mybir.dt.float32r, isOutput=False)
    fT = nc.declare_dram_parameter("fT", [D, NSH], mybir.dt.float32r, isOutput=False)
    out_vals = nc.declare_dram_parameter(
        "vals", [B, NBLOCKS * TOPB], mybir.dt.float32, isOutput=True
    )
    out_idx = nc.declare_dram_parameter(
        "idx", [B, NBLOCKS * TOPB], mybir.dt.uint32, isOutput=True
    )

    qT_r = qT.ap().rearrange("(k p) m -> p k m", p=128)
    fT_r = fT.ap().rearrange("(k p) n -> p k n", p=128)

    with TileContext(nc) as tc:
        with (
            tc.tile_pool(name="qpool", bufs=1) as qpool,
            tc.tile_pool(name="fpool", bufs=3) as fpool,
            tc.tile_pool(name="simspool", bufs=2) as simspool,
            tc.tile_pool(name="outpool", bufs=1) as outpool,
            tc.tile_pool(name="psum", bufs=4, space="PSUM") as psump,
        ):
            q_sb = qpool.tile([128, KC, B], mybir.dt.float32r)
            nc.sync.dma_start(out=q_sb[:], in_=qT_r)

            vals_st = outpool.tile([B, NBLOCKS * TOPB], mybir.dt.float32)
            idx_st = outpool.tile([B, NBLOCKS * TOPB], mybir.dt.uint32)

            for blk in range(NBLOCKS):
                c0 = blk * BLOCK_CHUNKS
                bchunks = min(BLOCK_CHUNKS, NCHUNKS - c0)
                bsize = bchunks * CHUNK
                sims = simspool.tile([B, BLOCK_CHUNKS * CHUNK], mybir.dt.float32)
                for g0 in range(0, bchunks, GRP):
                    gchunks = min(GRP, bchunks - g0)
                    gsize = gchunks * CHUNK
                    gstart = (c0 + g0) * CHUNK
                    f_sb = fpool.tile([128, KC, GRP * CHUNK], mybir.dt.float32r)
                    nc.sync.dma_start(
                        out=f_sb[:, :, :gsize],
                        in_=fT_r[:, :, gstart:gstart + gsize],
                    )
                    for c in range(gchunks):
                        ps = psump.tile([B, CHUNK], mybir.dt.float32)
                        for k in range(KC):
                            nc.tensor.matmul(
                                ps[:],
                                lhsT=q_sb[:, k, :],
                                rhs=f_sb[:, k, c * CHUNK:(c + 1) * CHUNK],
                                start=(k == 0),
                                stop=(k == KC - 1),
                            )
                        off = (g0 + c) * CHUNK
                        nc.scalar.copy(out=sims[:, off:off + CHUNK], in_=ps[:])
                nc.vector.max(
                    out=vals_st[:, blk * TOPB:(blk + 1) * TOPB],
                    in_=sims[:, :bsize],
                )
                nc.vector.max_index(
                    out=idx_st[:, blk * TOPB:(blk + 1) * TOPB],
                    in_max=vals_st[:, blk * TOPB:(blk + 1) * TOPB],
                    in_values=sims[:, :bsize],
                )
            nc.sync.dma_start(out=out_vals.ap(), in_=vals_st[:])
            nc.sync.dma_start(out=out_idx.ap(), in_=idx_st[:])

    nc.compile()
    return nc


def _get_compiled():
    global _COMPILED
    if _COMPILED is None:
        _COMPILED = _build()
    return _COMPILED


def kernel(query_feature, feature, data, k=5, **kwargs):
    global LAST_RESULTS
    q = np.ascontiguousarray(np.asarray(query_feature, dtype=np.float32))
    f = np.asarray(feature, dtype=np.float32)
    data = np.asarray(data)
    k = int(k)
    assert q.shape == (B, D) and f.shape == (N, D)

    nc = _get_compiled()

    qT = np.ascontiguousarray(q.T)
    in_maps = []
    for i in range(NCORES):
        fT = np.ascontiguousarray(f[i * NSH:(i + 1) * NSH].T)
        in_maps.append({"qT": qT, "fT": fT})

    res = run_bass_kernel_spmd(nc, in_maps, core_ids=list(range(NCORES)))
    LAST_RESULTS = res

    # Map block-local Max8 indices to global feature row indices.
    slot_block_off = (np.arange(NBLOCKS * TOPB) // TOPB) * (BLOCK_CHUNKS * CHUNK)
    cand = []
    for i in range(NCORES):
        idx = res.results[i]["idx"].astype(np.int64)  # (B, NBLOCKS*TOPB)
        cand.append(i * NSH + slot_block_off[None, :] + idx)
    cand = np.concatenate(cand, axis=1)  # (B, NCORES*NBLOCKS*TOPB)

    # Exact fp32 rescore of candidates (same math as the reference).
    qn = q / np.linalg.norm(q, axis=1, keepdims=True)
    fc = f[cand]  # (B, C, D)
    fn = fc / np.linalg.norm(fc, axis=2, keepdims=True)
    sims = np.einsum("bd,bcd->bc", qn, fn)  # fp32

    # Final top-k with jax.lax.top_k tie-breaking (value desc, index asc).
    o = np.argsort(cand, axis=1, kind="stable")
    cand_s = np.take_along_axis(cand, o, axis=1)
    sims_s = np.take_along_axis(sims, o, axis=1)
    sel = np.argsort(-sims_s, axis=1, kind="stable")[:, :k]
    top_idx = np.take_along_axis(cand_s, sel, axis=1)  # (B, k)

    return data[top_idx]  # (B, k, data_cols), input dtype preserved


# revision 4
# speedup vs baseline: 1.8515x; 1.8515x over previous
"""Distributed brute-force kNN retrieval (cosine similarity) on 8 Trainium2 cores.

Strategy (per spec sharding hint, adapted):
  - Shard the feature bank along N across 8 cores (62500 rows each).
  - Host pre-transposes each shard to [768, 62500] so the device can DMA
    contraction-major tiles directly (no on-device transpose needed).
  - Each core computes raw dot products q @ f_shard.T with float32r matmuls
    (PE-native, ~1e-4 relative error) and extracts the top-8 candidates per
    query per 8000-feature block with the DVE Max8/MaxIndex instructions.
  - Host gathers the 8*64 candidates per (query, core), rescores them exactly
    in fp32 (normalized cosine similarity, same math as the reference), does
    the final top-k reduction, and gathers the data segments.

The candidate margin (top-8 of every 8000-feature block when only the global
top-5 is needed) makes the device pass insensitive to the reduced-precision
matmul: a true top-5 entry would have to be pushed below rank 8 *within its
own block* by ~1e-2-magnitude noise on dot gaps that are ~1e0 apart.
"""

import ml_dtypes
import numpy as np

import concourse.bacc as bacc
import concourse.mybir as mybir
from concourse.tile import TileContext
from concourse.bass_utils import run_bass_kernel_spmd

# Problem geometry (hardcoded per spec).
B = 64            # queries
D = 768           # feature dim
N = 500000        # feature rows
NCORES = 8
NSH = N // NCORES  # 62500 rows per core
KC = D // 128      # 6 contraction chunks of 128
CHUNK = 500        # matmul moving free dim (one PSUM bank, >=256 keeps fp32r fast)
NCHUNKS = NSH // CHUNK          # 125
GRP = 2                         # chunks per DMA group (3 MB per DMA)
BLOCK_CHUNKS = 16               # chunks per Max8 block (8000 features)
NBLOCKS = (NCHUNKS + BLOCK_CHUNKS - 1) // BLOCK_CHUNKS  # 8 (7 full + 13-chunk tail)
TOPB = 8                        # Max8 output width per block

_COMPILED = None
LAST_RESULTS = None  # test harness introspection


def _build():
    nc = bacc.Bacc("TRN2", target_bir_lowering=False, debug=False)
    qT = nc.declare_dram_parameter("qT", [D, B], mybir.dt.bfloat16, isOutput=False)
    fT = nc.declare_dram_parameter("fT", [D, NSH], mybir.dt.bfloat16, isOutput=False)
    out_vals = nc.declare_dram_parameter(
        "vals", [B, NBLOCKS * TOPB], mybir.dt.float32, isOutput=True
    )
    out_idx = nc.declare_dram_parameter(
        "idx", [B, NBLOCKS * TOPB], mybir.dt.uint32, isOutput=True
    )

    qT_r = qT.ap().rearrange("(k p) m -> p k m", p=128)
    fT_r = fT.ap().rearrange("(k p) n -> p k n", p=128)

    with TileContext(nc) as tc:
        with (
            tc.tile_pool(name="qpool", bufs=1) as qpool,
            tc.tile_pool(name="fpool", bufs=3) as fpool,
            tc.tile_pool(name="simspool", bufs=2) as simspool,
            tc.tile_pool(name="outpool", bufs=1) as outpool,
            tc.tile_pool(name="psum", bufs=4, space="PSUM") as psump,
        ):
            q_sb = qpool.tile([128, KC, B], mybir.dt.bfloat16)
            nc.sync.dma_start(out=q_sb[:], in_=qT_r)

            vals_st = outpool.tile([B, NBLOCKS * TOPB], mybir.dt.float32)
            idx_st = outpool.tile([B, NBLOCKS * TOPB], mybir.dt.uint32)

            for blk in range(NBLOCKS):
                c0 = blk * BLOCK_CHUNKS
                bchunks = min(BLOCK_CHUNKS, NCHUNKS - c0)
                bsize = bchunks * CHUNK
                sims = simspool.tile([B, BLOCK_CHUNKS * CHUNK], mybir.dt.float32)
                for g0 in range(0, bchunks, GRP):
                    gchunks = min(GRP, bchunks - g0)
                    gsize = gchunks * CHUNK
                    gstart = (c0 + g0) * CHUNK
                    f_sb = fpool.tile([128, KC, GRP * CHUNK], mybir.dt.bfloat16)
                    nc.sync.dma_start(
                        out=f_sb[:, :, :gsize],
                        in_=fT_r[:, :, gstart:gstart + gsize],
                    )
                    for c in range(gchunks):
                        ps = psump.tile([B, CHUNK], mybir.dt.float32)
                        for k in range(KC):
                            nc.tensor.matmul(
                                ps[:],
                                lhsT=q_sb[:, k, :],
                                rhs=f_sb[:, k, c * CHUNK:(c + 1) * CHUNK],
                                start=(k == 0),
                                stop=(k == KC - 1),
                            )
                        off = (g0 + c) * CHUNK
                        nc.scalar.copy(out=sims[:, off:off + CHUNK], in_=ps[:])
                nc.vector.max(
                    out=vals_st[:, blk * TOPB:(blk + 1) * TOPB],
                    in_=sims[:, :bsize],
                )
                nc.vector.max_index(
                    out=idx_st[:, blk * TOPB:(blk + 1) * TOPB],
                    in_max=vals_st[:, blk * TOPB:(blk + 1) * TOPB],
                    in_values=sims[:, :bsize],
                )
            nc.sync.dma_start(out=out_vals.ap(), in_=vals_st[:])
            nc.sync.dma_start(out=out_idx.ap(), in_=idx_st[:])

    nc.compile()
    return nc


def _get_compiled():
    global _COMPILED
    if _COMPILED is None:
        _COMPILED = _build()
    return _COMPILED


def kernel(query_feature, feature, data, k=5, **kwargs):
    global LAST_RESULTS
    q = np.ascontiguousarray(np.asarray(query_feature, dtype=np.float32))
    f = np.asarray(feature, dtype=np.float32)
    data = np.asarray(data)
    k = int(k)
    assert q.shape == (B, D) and f.shape == (N, D)

    nc = _get_compiled()

    qT = np.ascontiguousarray(q.T.astype(ml_dtypes.bfloat16))
    in_maps = []
    for i in range(NCORES):
        fT = np.ascontiguousarray(f[i * NSH:(i + 1) * NSH].T.astype(ml_dtypes.bfloat16))
        in_maps.append({"qT": qT, "fT": fT})

    res = run_bass_kernel_spmd(nc, in_maps, core_ids=list(range(NCORES)))
    LAST_RESULTS = res

    # Map block-local Max8 indices to global feature row indices.
    slot_block_off = (np.arange(NBLOCKS * TOPB) // TOPB) * (BLOCK_CHUNKS * CHUNK)
    cand = []
    for i in range(NCORES):
        idx = res.results[i]["idx"].astype(np.int64)  # (B, NBLOCKS*TOPB)
        cand.append(i * NSH + slot_block_off[None, :] + idx)
    cand = np.concatenate(cand, axis=1)  # (B, NCORES*NBLOCKS*TOPB)

    # Exact fp32 rescore of candidates (same math as the reference).
    qn = q / np.linalg.norm(q, axis=1, keepdims=True)
    fc = f[cand]  # (B, C, D)
    fn = fc / np.linalg.norm(fc, axis=2, keepdims=True)
    sims = np.einsum("bd,bcd->bc", qn, fn)  # fp32

    # Final top-k with jax.lax.top_k tie-breaking (value desc, index asc).
    o = np.argsort(cand, axis=1, kind="stable")
    cand_s = np.take_along_axis(cand, o, axis=1)
    sims_s = np.take_along_axis(sims, o, axis=1)
    sel = np.argsort(-sims_s, axis=1, kind="stable")[:, :k]
    top_idx = np.take_along_axis(cand_s, sel, axis=1)  # (B, k)

    return data[top_idx]  # (B, k, data_cols), input dtype preserved


# revision 5
# speedup vs baseline: 1.9445x; 1.0502x over previous
"""Distributed brute-force kNN retrieval (cosine similarity) on 8 Trainium2 cores.

Strategy (per spec sharding hint, adapted):
  - Shard the feature bank along N across 8 cores (62500 rows each).
  - Host pre-transposes each shard to [768, 62500] (bf16) so the device can
    DMA contraction-major tiles directly (no on-device transpose needed).
  - Each core computes raw dot products q @ f_shard.T with bf16 matmuls
    (fp32 PSUM accumulation). Feature chunks are processed two-at-a-time via
    PE column tiling (tile_position=(0,64)): chunk A lands on PSUM partitions
    0-63, chunk B on 64-127, so all 128 partitions carry similarities.
  - The DVE Max8/MaxIndex instructions extract the top-8 candidates per
    query-row per similarity block.
  - Host maps candidates to global row indices, rescores them exactly in fp32
    (normalized cosine similarity, same math as the reference), does the final
    top-k reduction, and gathers the data segments.

The candidate margin (top-8 of every 4000-feature half-block when only the
global top-5 is needed) makes the device pass insensitive to bf16 rounding:
a true top-5 entry would have to be pushed below rank 8 *within its own
block* by ~0.05-sigma noise on dot gaps that are many sigma apart.
"""

import ml_dtypes
import numpy as np

import concourse.bacc as bacc
import concourse.mybir as mybir
from concourse.tile import TileContext
from concourse.bass_utils import run_bass_kernel_spmd

# Problem geometry (hardcoded per spec).
B = 64             # queries
D = 768            # feature dim
N = 500000         # feature rows
NCORES = 8
NSH = N // NCORES  # 62500 rows per core
KC = D // 128      # 6 contraction chunks of 128
CHUNK = 500        # matmul moving free dim (one PSUM bank)
NCHUNKS = NSH // CHUNK           # 125 chunks; 62 pairs + 1 solo chunk
NPAIRS = NCHUNKS // 2            # 62
BLOCK_PAIRS = 8                  # pairs per Max8 block (8000 features)
NPBLOCKS = (NPAIRS + BLOCK_PAIRS - 1) // BLOCK_PAIRS  # 8 (7 full + 6-pair tail)
NBLOCKS = NPBLOCKS + 1           # + solo-chunk block
GRP = 4                          # chunks per DMA group (3 MB bf16)
TOPB = 8                         # Max8 output width per block

_COMPILED = None
LAST_RESULTS = None  # test harness introspection


def _build():
    nc = bacc.Bacc("TRN2", target_bir_lowering=False, debug=False)
    qT = nc.declare_dram_parameter("qT", [D, B], mybir.dt.bfloat16, isOutput=False)
    fT = nc.declare_dram_parameter("fT", [D, NSH], mybir.dt.bfloat16, isOutput=False)
    out_vals = nc.declare_dram_parameter(
        "vals", [128, NBLOCKS * TOPB], mybir.dt.float32, isOutput=True
    )
    out_idx = nc.declare_dram_parameter(
        "idx", [128, NBLOCKS * TOPB], mybir.dt.uint32, isOutput=True
    )

    qT_r = qT.ap().rearrange("(k p) m -> p k m", p=128)
    fT_r = fT.ap().rearrange("(k p) n -> p k n", p=128)

    with TileContext(nc) as tc:
        with (
            tc.tile_pool(name="qpool", bufs=1) as qpool,
            tc.tile_pool(name="fpool", bufs=4) as fpool,
            tc.tile_pool(name="simspool", bufs=2) as simspool,
            tc.tile_pool(name="outpool", bufs=1) as outpool,
            tc.tile_pool(name="psum", bufs=4, space="PSUM") as psump,
        ):
            q_sb = qpool.tile([128, KC, B], mybir.dt.bfloat16)
            nc.sync.dma_start(out=q_sb[:], in_=qT_r)

            vals_st = outpool.tile([128, NBLOCKS * TOPB], mybir.dt.float32)
            idx_st = outpool.tile([128, NBLOCKS * TOPB], mybir.dt.uint32)

            # DMA group chunk-starts: 31 groups of 4 chunks + 1 solo chunk.
            f_tiles = {}

            def load_group(gc0):
                gchunks = min(GRP, NCHUNKS - gc0)
                f_sb = fpool.tile([128, KC, GRP * CHUNK], mybir.dt.bfloat16)
                nc.sync.dma_start(
                    out=f_sb[:, :, :gchunks * CHUNK],
                    in_=fT_r[:, :, gc0 * CHUNK:(gc0 + gchunks) * CHUNK],
                )
                for c in range(gchunks):
                    f_tiles[gc0 + c] = f_sb[:, :, c * CHUNK:(c + 1) * CHUNK]

            def mm_pair(ps, ca, cb):
                # chunk ca -> PSUM partitions 0:64, chunk cb -> 64:128
                for k in range(KC):
                    nc.tensor.matmul(
                        ps[0:B, :],
                        lhsT=q_sb[:, k, :],
                        rhs=f_tiles[ca][:, k, :],
                        start=(k == 0),
                        stop=(k == KC - 1),
                    )
                if cb is not None:
                    for k in range(KC):
                        nc.tensor.matmul(
                            ps[B:2 * B, :],
                            lhsT=q_sb[:, k, :],
                            rhs=f_tiles[cb][:, k, :],
                            start=(k == 0),
                            stop=(k == KC - 1),
                            tile_position=(0, B),
                        )

            for blk in range(NPBLOCKS):
                p0 = blk * BLOCK_PAIRS
                bpairs = min(BLOCK_PAIRS, NPAIRS - p0)
                bsize = bpairs * CHUNK
                sims = simspool.tile([128, BLOCK_PAIRS * CHUNK], mybir.dt.float32)
                for j in range(bpairs):
                    ca, cb = 2 * (p0 + j), 2 * (p0 + j) + 1
                    if ca % GRP == 0:
                        load_group(ca)
                    ps = psump.tile([128, CHUNK], mybir.dt.float32)
                    mm_pair(ps, ca, cb)
                    nc.scalar.copy(
                        out=sims[:, j * CHUNK:(j + 1) * CHUNK], in_=ps[:]
                    )
                nc.vector.max(
                    out=vals_st[:, blk * TOPB:(blk + 1) * TOPB],
                    in_=sims[:, :bsize],
                )
                nc.vector.max_index(
                    out=idx_st[:, blk * TOPB:(blk + 1) * TOPB],
                    in_max=vals_st[:, blk * TOPB:(blk + 1) * TOPB],
                    in_values=sims[:, :bsize],
                )

            # Solo tail chunk (124) -> its own block on partitions 0:64.
            solo = NCHUNKS - 1
            load_group(solo)
            sims = simspool.tile([128, BLOCK_PAIRS * CHUNK], mybir.dt.float32)
            ps = psump.tile([128, CHUNK], mybir.dt.float32)
            mm_pair(ps, solo, None)
            nc.scalar.copy(out=sims[0:B, 0:CHUNK], in_=ps[0:B, :])
            nc.vector.max(
                out=vals_st[0:B, NPBLOCKS * TOPB:(NPBLOCKS + 1) * TOPB],
                in_=sims[0:B, :CHUNK],
            )
            nc.vector.max_index(
                out=idx_st[0:B, NPBLOCKS * TOPB:(NPBLOCKS + 1) * TOPB],
                in_max=vals_st[0:B, NPBLOCKS * TOPB:(NPBLOCKS + 1) * TOPB],
                in_values=sims[0:B, :CHUNK],
            )

            nc.sync.dma_start(out=out_vals.ap(), in_=vals_st[:])
            nc.sync.dma_start(out=out_idx.ap(), in_=idx_st[:])

    nc.compile()
    return nc


def _get_compiled():
    global _COMPILED
    if _COMPILED is None:
        _COMPILED = _build()
    return _COMPILED


def _candidate_indices(idx_arr):
    """Map device Max8 indices (128, NBLOCKS*8) to shard-local feature rows.

    Row p < 64 is query p over even chunks of each pair-block; row p >= 64 is
    query p-64 over odd chunks. Block b covers pairs [b*8, b*8+bpairs); a
    Max8 index i within the block means pair j = i//500, pos = i%500, i.e.
    chunk 2*(b*8 + j) + half, feature row = chunk*500 + pos.
    """
    out = []
    for q in range(B):
        rows = []
        for half in (0, 1):
            v = idx_arr[q + half * B].astype(np.int64)  # (NBLOCKS*TOPB,)
            for blk in range(NPBLOCKS):
                i = v[blk * TOPB:(blk + 1) * TOPB]
                chunk = 2 * (blk * BLOCK_PAIRS + i // CHUNK) + half
                rows.append(chunk * CHUNK + i % CHUNK)
        # solo block: only half 0, chunk NCHUNKS-1
        i = idx_arr[q, NPBLOCKS * TOPB:(NPBLOCKS + 1) * TOPB].astype(np.int64)
        rows.append((NCHUNKS - 1) * CHUNK + i)
        out.append(np.concatenate(rows))
    return np.stack(out)  # (B, (2*NPBLOCKS+1)*TOPB)


def kernel(query_feature, feature, data, k=5, **kwargs):
    global LAST_RESULTS
    q = np.ascontiguousarray(np.asarray(query_feature, dtype=np.float32))
    f = np.asarray(feature, dtype=np.float32)
    data = np.asarray(data)
    k = int(k)
    assert q.shape == (B, D) and f.shape == (N, D)

    nc = _get_compiled()

    qT = np.ascontiguousarray(q.T.astype(ml_dtypes.bfloat16))
    in_maps = []
    for i in range(NCORES):
        fT = np.ascontiguousarray(
            f[i * NSH:(i + 1) * NSH].T.astype(ml_dtypes.bfloat16)
        )
        in_maps.append({"qT": qT, "fT": fT})

    res = run_bass_kernel_spmd(nc, in_maps, core_ids=list(range(NCORES)))
    LAST_RESULTS = res

    cand = np.concatenate(
        [
            i * NSH + _candidate_indices(res.results[i]["idx"])
            for i in range(NCORES)
        ],
        axis=1,
    )  # (B, NCORES * (2*NPBLOCKS+1) * TOPB)

    # Exact fp32 rescore of candidates (same math as the reference).
    qn = q / np.linalg.norm(q, axis=1, keepdims=True)
    fc = f[cand]  # (B, C, D)
    fn = fc / np.linalg.norm(fc, axis=2, keepdims=True)
    sims = np.einsum("bd,bcd->bc", qn, fn)  # fp32

    # Final top-k with jax.lax.top_k tie-breaking (value desc, index asc).
    o = np.argsort(cand, axis=1, kind="stable")
    cand_s = np.take_along_axis(cand, o, axis=1)
    sims_s = np.take_along_axis(sims, o, axis=1)
    sel = np.argsort(-sims_s, axis=1, kind="stable")[:, :k]
    top_idx = np.take_along_axis(cand_s, sel, axis=1)  # (B, k)

    return data[top_idx]  # (B, k, data_cols), input dtype preserved


# revision 6
# speedup vs baseline: 4.0524x; 2.0841x over previous
"""Distributed brute-force kNN retrieval (cosine similarity) on 8 Trainium2 cores.

Strategy (per spec sharding hint, adapted):
  - Shard the feature bank along N across 8 cores (62500 rows each).
  - Host pre-transposes each shard to [768, 62500] (bf16) so the device can
    DMA contraction-major tiles directly (no on-device transpose needed).
  - Each core computes raw dot products q @ f_shard.T with bf16 matmuls
    (fp32 PSUM accumulation). Feature chunks are processed two-at-a-time via
    PE column tiling (tile_position=(0,64)): chunk A lands on PSUM partitions
    0-63, chunk B on 64-127, so all 128 partitions carry similarities.
  - The DVE Max8/MaxIndex instructions extract the top-8 candidates per
    query-row per similarity block.
  - Host maps candidates to global row indices, rescores them exactly in fp32
    (normalized cosine similarity, same math as the reference), does the final
    top-k reduction, and gathers the data segments.

The candidate margin (top-8 of every 4000-feature half-block when only the
global top-5 is needed) makes the device pass insensitive to bf16 rounding:
a true top-5 entry would have to be pushed below rank 8 *within its own
block* by ~0.05-sigma noise on dot gaps that are many sigma apart.
"""

import ml_dtypes
import numpy as np

import concourse.bacc as bacc
import concourse.mybir as mybir
from concourse.tile import TileContext
from concourse.bass_utils import run_bass_kernel_spmd

# Problem geometry (hardcoded per spec).
B = 64             # queries
D = 768            # feature dim
N = 500000         # feature rows
NCORES = 8
NSH = N // NCORES  # 62500 rows per core
KC = D // 128      # 6 contraction chunks of 128
CHUNK = 500        # matmul moving free dim (one PSUM bank)
NCHUNKS = NSH // CHUNK           # 125 chunks; 62 pairs + 1 solo chunk
NPAIRS = NCHUNKS // 2            # 62
BLOCK_PAIRS = 8                  # pairs per Max8 block (8000 features)
NPBLOCKS = (NPAIRS + BLOCK_PAIRS - 1) // BLOCK_PAIRS  # 8 (7 full + 6-pair tail)
NBLOCKS = NPBLOCKS + 1           # + solo-chunk block
GRP = 8                          # chunks per DMA group (3 MB fp8)
TOPB = 8                         # Max8 output width per block

_COMPILED = None
LAST_RESULTS = None  # test harness introspection


def _build():
    nc = bacc.Bacc("TRN2", target_bir_lowering=False, debug=False)
    qT = nc.declare_dram_parameter("qT", [D, B], mybir.dt.float8e4, isOutput=False)
    fT = nc.declare_dram_parameter("fT", [D, NSH], mybir.dt.float8e4, isOutput=False)
    out_vals = nc.declare_dram_parameter(
        "vals", [128, NBLOCKS * TOPB], mybir.dt.float32, isOutput=True
    )
    out_idx = nc.declare_dram_parameter(
        "idx", [128, NBLOCKS * TOPB], mybir.dt.uint32, isOutput=True
    )

    qT_r = qT.ap().rearrange("(k p) m -> p k m", p=128)
    fT_r = fT.ap().rearrange("(k p) n -> p k n", p=128)

    with TileContext(nc) as tc:
        with (
            tc.tile_pool(name="qpool", bufs=1) as qpool,
            tc.tile_pool(name="fpool", bufs=4) as fpool,
            tc.tile_pool(name="simspool", bufs=2) as simspool,
            tc.tile_pool(name="outpool", bufs=1) as outpool,
            tc.tile_pool(name="psum", bufs=4, space="PSUM") as psump,
        ):
            q_sb = qpool.tile([128, KC, B], mybir.dt.float8e4)
            nc.sync.dma_start(out=q_sb[:], in_=qT_r)

            vals_st = outpool.tile([128, NBLOCKS * TOPB], mybir.dt.float32)
            idx_st = outpool.tile([128, NBLOCKS * TOPB], mybir.dt.uint32)

            # DMA group chunk-starts: 31 groups of 4 chunks + 1 solo chunk.
            f_tiles = {}

            def load_group(gc0):
                gchunks = min(GRP, NCHUNKS - gc0)
                f_sb = fpool.tile([128, KC, GRP * CHUNK], mybir.dt.float8e4)
                nc.sync.dma_start(
                    out=f_sb[:, :, :gchunks * CHUNK],
                    in_=fT_r[:, :, gc0 * CHUNK:(gc0 + gchunks) * CHUNK],
                )
                for c in range(gchunks):
                    f_tiles[gc0 + c] = f_sb[:, :, c * CHUNK:(c + 1) * CHUNK]

            def mm_pair(ps, ca, cb):
                # chunk ca -> PSUM partitions 0:64, chunk cb -> 64:128
                for k in range(KC):
                    nc.tensor.matmul(
                        ps[0:B, :],
                        lhsT=q_sb[:, k, :],
                        rhs=f_tiles[ca][:, k, :],
                        start=(k == 0),
                        stop=(k == KC - 1),
                    )
                if cb is not None:
                    for k in range(KC):
                        nc.tensor.matmul(
                            ps[B:2 * B, :],
                            lhsT=q_sb[:, k, :],
                            rhs=f_tiles[cb][:, k, :],
                            start=(k == 0),
                            stop=(k == KC - 1),
                            tile_position=(0, B),
                        )

            for blk in range(NPBLOCKS):
                p0 = blk * BLOCK_PAIRS
                bpairs = min(BLOCK_PAIRS, NPAIRS - p0)
                bsize = bpairs * CHUNK
                sims = simspool.tile([128, BLOCK_PAIRS * CHUNK], mybir.dt.float32)
                for j in range(bpairs):
                    ca, cb = 2 * (p0 + j), 2 * (p0 + j) + 1
                    if ca % GRP == 0:
                        load_group(ca)
                    ps = psump.tile([128, CHUNK], mybir.dt.float32)
                    mm_pair(ps, ca, cb)
                    nc.scalar.copy(
                        out=sims[:, j * CHUNK:(j + 1) * CHUNK], in_=ps[:]
                    )
                nc.vector.max(
                    out=vals_st[:, blk * TOPB:(blk + 1) * TOPB],
                    in_=sims[:, :bsize],
                )
                nc.vector.max_index(
                    out=idx_st[:, blk * TOPB:(blk + 1) * TOPB],
                    in_max=vals_st[:, blk * TOPB:(blk + 1) * TOPB],
                    in_values=sims[:, :bsize],
                )

            # Solo tail chunk (124) -> its own block on partitions 0:64.
            solo = NCHUNKS - 1
            load_group(solo)
            sims = simspool.tile([128, BLOCK_PAIRS * CHUNK], mybir.dt.float32)
            ps = psump.tile([128, CHUNK], mybir.dt.float32)
            mm_pair(ps, solo, None)
            nc.scalar.copy(out=sims[0:B, 0:CHUNK], in_=ps[0:B, :])
            nc.vector.max(
                out=vals_st[0:B, NPBLOCKS * TOPB:(NPBLOCKS + 1) * TOPB],
                in_=sims[0:B, :CHUNK],
            )
            nc.vector.max_index(
                out=idx_st[0:B, NPBLOCKS * TOPB:(NPBLOCKS + 1) * TOPB],
                in_max=vals_st[0:B, NPBLOCKS * TOPB:(NPBLOCKS + 1) * TOPB],
                in_values=sims[0:B, :CHUNK],
            )

            nc.sync.dma_start(out=out_vals.ap(), in_=vals_st[:])
            nc.sync.dma_start(out=out_idx.ap(), in_=idx_st[:])

    nc.compile()
    return nc


def _get_compiled():
    global _COMPILED
    if _COMPILED is None:
        _COMPILED = _build()
    return _COMPILED


def _candidate_indices(idx_arr):
    """Map device Max8 indices (128, NBLOCKS*8) to shard-local feature rows.

    Row p < 64 is query p over even chunks of each pair-block; row p >= 64 is
    query p-64 over odd chunks. Block b covers pairs [b*8, b*8+bpairs); a
    Max8 index i within the block means pair j = i//500, pos = i%500, i.e.
    chunk 2*(b*8 + j) + half, feature row = chunk*500 + pos.
    """
    out = []
    for q in range(B):
        rows = []
        for half in (0, 1):
            v = idx_arr[q + half * B].astype(np.int64)  # (NBLOCKS*TOPB,)
            for blk in range(NPBLOCKS):
                i = v[blk * TOPB:(blk + 1) * TOPB]
                chunk = 2 * (blk * BLOCK_PAIRS + i // CHUNK) + half
                rows.append(chunk * CHUNK + i % CHUNK)
        # solo block: only half 0, chunk NCHUNKS-1
        i = idx_arr[q, NPBLOCKS * TOPB:(NPBLOCKS + 1) * TOPB].astype(np.int64)
        rows.append((NCHUNKS - 1) * CHUNK + i)
        out.append(np.concatenate(rows))
    return np.stack(out)  # (B, (2*NPBLOCKS+1)*TOPB)


def kernel(query_feature, feature, data, k=5, **kwargs):
    global LAST_RESULTS
    q = np.ascontiguousarray(np.asarray(query_feature, dtype=np.float32))
    f = np.asarray(feature, dtype=np.float32)
    data = np.asarray(data)
    k = int(k)
    assert q.shape == (B, D) and f.shape == (N, D)

    nc = _get_compiled()

    F8 = mybir.dt.np(mybir.dt.float8e4)
    qT = np.ascontiguousarray(q.T.astype(F8))
    in_maps = []
    for i in range(NCORES):
        fT = np.ascontiguousarray(
            f[i * NSH:(i + 1) * NSH].T.astype(F8)
        )
        in_maps.append({"qT": qT, "fT": fT})

    res = run_bass_kernel_spmd(nc, in_maps, core_ids=list(range(NCORES)))
    LAST_RESULTS = res

    cand = np.concatenate(
        [
            i * NSH + _candidate_indices(res.results[i]["idx"])
            for i in range(NCORES)
        ],
        axis=1,
    )  # (B, NCORES * (2*NPBLOCKS+1) * TOPB)

    # Exact fp32 rescore of candidates (same math as the reference).
    qn = q / np.linalg.norm(q, axis=1, keepdims=True)
    fc = f[cand]  # (B, C, D)
    fn = fc / np.linalg.norm(fc, axis=2, keepdims=True)
    sims = np.einsum("bd,bcd->bc", qn, fn)  # fp32

    # Final top-k with jax.lax.top_k tie-breaking (value desc, index asc).
    o = np.argsort(cand, axis=1, kind="stable")
    cand_s = np.take_along_axis(cand, o, axis=1)
    sims_s = np.take_along_axis(sims, o, axis=1)
    sel = np.argsort(-sims_s, axis=1, kind="stable")[:, :k]
    top_idx = np.take_along_axis(cand_s, sel, axis=1)  # (B, k)

    return data[top_idx]  # (B, k, data_cols), input dtype preserved
